# revision 32
# baseline (speedup 1.0000x reference)
"""LlamaAttention (B=2, S=2048, D=2048, H=16) on 8 Trainium2 NeuronCores.

Sharding: batch x head-group. Core c handles batch b = c // 4 and head group
g = c % 4 (4 heads of 128 dims each -> a 512-wide slice of q/k/v space).
Each core computes q/k/v projections for its slice, attention for its 4
heads, and a partial out-projection (contracting only its 512 dv dims).
Host sums the 4 partials per batch and adds the output bias.

Design (measured ~365us vs the 694us v1 baseline; rel err ~5.4e-3 vs 2e-2):
  - All matmul data is bf16 (same PE rate as fp32r, half the DMA/SBUF);
    accumulation is fp32 in PSUM, softmax denominators are exact fp32.
  - One x pass computes q/k/v: packed [D,3E] weight tiles (3KB DMA lines),
    x in [128,1024] tiles (2KB lines), chunk-0 x DMAs interleaved with
    weight DMAs so the first matmul issues after ~2 tile loads. q/k
    accumulate dt-major in 4 PSUM banks (two et-halves); SCALE is folded
    into Wq on the host.
  - Attention uses the transposed softmax layout (keys on partitions):
    scores^T tiles -> exp on ScalarE straight out of PSUM. With the
    all-ones mask two score matmuls share one [128,1024] 2-bank PSUM tile
    and a single exp (8 ScalarE ops/head); a nontrivial mask switches to
    the per-key-tile variant with the mask as the activation's
    per-partition bias (exact general masking).
  - Row sums (softmax denominator) ride a 7-op elementwise [128,1024]
    bf16 add tree on the DVE; one all-ones [128,128] stationary matmul
    per head then partition-reduces AND broadcasts the sum across all
    128 partitions in one shot, so normalization is reciprocal_approx_fast
    + one tensor_mul (no GpSimd, no per-partition broadcast).
  - scores/PV matmuls interleave at key-tile granularity; the previous
    block's out-projection is emitted between blocks. Engine assignment:
    exp + q/v evictions on ScalarE, k/y evictions + tree + normalize on
    DVE; the final block's out-projection alternates evict engines.
  - q/k PSUM accumulators rotate over 6 tags so a chunk-half's first
    matmul never waits the previous half's eviction.
  - The q projections of s-chunks 3 and 2 are deferred into attention
    blocks 0 and 1 (their readers - blocks 3 and 2's scores - run much
    later): those heads otherwise idle the PE while exp paces them, so
    the 128 deferred matmuls ride in the idle slots, shortening the
    projection phase by ~28us. Each accumulator borrows the out-proj
    PSUM tag during a window where it is provably idle (out-proj of
    block b-1 executes only after block b's heads). The operands are
    dedicated persistent SBUF copies: the x superchunk-1 copy doubles as
    the projection's own input, so the xp pool drops to single-buffered
    and superchunk 1 prefetches with no write-after-read wait at all.
  - Head 0 of blocks 2 and 3 is pre-computed: its scores/exp/tree are
    emitted ahead of the preceding block's out-projection, so ScalarE
    chews those exps during the otherwise scalar-idle out-proj window
    and the head body reduces to a pure PV burst.
  - Known wall: the attention phase rides the chip power limit. Tighter
    interleavings (out-proj inside heads, deferred r-chain) measurably
    tripped a chip-wide ~1.2x downclock (P0: matmuls 379->454ns, exp
    1112->1334ns), netting slower kernels; the phase-alternating
    schedule stays at 2.4GHz and is within ~4% of the power-constrained
    optimum. ScalarE exp (~9us/head) vs PE (~7us/head + out-proj share)
    are nearly balanced there.
"""

import os
import numpy as np
import ml_dtypes

import concourse.bass as bass
import concourse.tile as tile
from concourse import bacc, mybir
from concourse import bass_utils

B, S, D = 2, 2048, 2048
NH, HD = 16, 128
N_CORES = 8
HPC = 4                      # heads per core
E = HPC * HD                 # 512: per-core q/k/v width
SCALE = float(HD) ** -0.5
F32 = mybir.dt.float32
BF16 = mybir.dt.bfloat16
NPBF16 = ml_dtypes.bfloat16

P = 128                      # partition tile
ST = S // P                  # 16 s partition-tiles
DTI = D // P                 # 16 d partition-tiles
SB = 512                     # matmul moving-dim block
NBLK = S // SB               # 4 s blocks
MASK_MIN = float(np.finfo(np.float32).min)

MM_DT = BF16                 # for test.py's printout


def _build(has_bias: bool, paired: bool):
    nc = bacc.Bacc("TRN2", target_bir_lowering=False, debug=False,
                   num_devices=N_CORES)

    xT = nc.dram_tensor("xT", [D, S], BF16, kind="ExternalInput").ap()
    wqkvT = nc.dram_tensor("wqkvT", [D, 3 * E], BF16,
                           kind="ExternalInput").ap()
    woT = nc.dram_tensor("woT", [E, D], BF16, kind="ExternalInput").ap()
    maskT = nc.dram_tensor("maskT", [S], F32, kind="ExternalInput").ap()
    if has_bias:
        ones1 = nc.dram_tensor("ones1", [SB], BF16, kind="ExternalInput").ap()
        bqd = nc.dram_tensor("bq", [E], BF16, kind="ExternalInput").ap()
        bkd = nc.dram_tensor("bk", [E], BF16, kind="ExternalInput").ap()
        bvd = nc.dram_tensor("bv", [E], BF16, kind="ExternalInput").ap()
    yT = nc.dram_tensor("yT", [D, S], BF16, kind="ExternalOutput").ap()

    with tile.TileContext(nc) as tc:
        with tc.tile_pool(name="persist", bufs=1) as persist:
            qT = [persist.tile([P, S], BF16, name=f"qT{i}", tag=f"qT{i}")
                  for i in range(HPC)]
            kT = [persist.tile([P, S], BF16, name=f"kT{i}", tag=f"kT{i}")
                  for i in range(HPC)]
            mask_sb = persist.tile([P, ST], F32, name="mask_sb", tag="mask")
            ones_sq = persist.tile([P, P], BF16, name="ones_sq", tag="onesq")
            scr = persist.tile([P, ST], BF16, name="scr", tag="scr")
            nc.vector.memset(ones_sq[:, :], 1.0)
            if has_bias:
                ones_row = persist.tile([1, SB], BF16, name="ones_row",
                                        tag="onesr")
                nc.sync.dma_start(ones_row[:, :],
                                  ones1.rearrange("(a e) -> a e", a=1))
                ones_rp = persist.tile([1, P], BF16, name="ones_rp",
                                       tag="onesrp")
                nc.sync.dma_start(ones_rp[:, :],
                                  ones1[0:P].rearrange("(a e) -> a e", a=1))
                bq_sb = persist.tile([1, E], BF16, name="bq_sb", tag="bq")
                bk_sb = persist.tile([1, E], BF16, name="bk_sb", tag="bk")
                bv_sb = persist.tile([1, E], BF16, name="bv_sb", tag="bv")
                nc.sync.dma_start(bq_sb[:, :], bqd.rearrange("(a e) -> a e", a=1))
                nc.sync.dma_start(bk_sb[:, :], bkd.rearrange("(a e) -> a e", a=1))
                nc.sync.dma_start(bv_sb[:, :], bvd.rearrange("(a e) -> a e", a=1))

            with tc.tile_pool(name="vvp", bufs=1) as vvp, \
                 tc.tile_pool(name="wop", bufs=1) as wop:
                vv = [vvp.tile([P, E], BF16, name=f"v{i}", tag=f"v{i}")
                      for i in range(ST)]
                wo_sb = [wop.tile([P, D], BF16, name=f"wo{i}", tag=f"wo{i}")
                         for i in range(HPC)]
                # dedicated copies of the deferred-q operands (w q-columns;
                # x superchunk 1 lives here outright and the projections read
                # it in place, so xp needs no double buffering for it)
                if paired:
                    wq3 = [vvp.tile([P, E], BF16, name=f"wq3_{i}",
                                    tag=f"wq3_{i}") for i in range(DTI)]
                    xq23 = [vvp.tile([P, 2 * SB], BF16, name=f"xq23_{i}",
                                     tag=f"xq23_{i}") for i in range(DTI)]

                # ---------------- projections: one x pass ----------------
                # x streams once in [128,1024] tiles (2KB DMA lines); the
                # packed wqkv weight tiles are [128,1536] (3KB lines).
                # Chunk-0 x DMAs interleave with weight DMAs so the first
                # matmul starts after ~2 tile loads, not the full weight set.
                XW = 2 * SB
                with nc.named_scope("proj"), \
                     tc.tile_pool(name="wqkv", bufs=1) as wp, \
                     tc.tile_pool(name="xp", bufs=2) as xp, \
                     tc.tile_pool(name="ps_p", bufs=1, space="PSUM") as psp, \
                     tc.tile_pool(name="ps_v", bufs=2, space="PSUM") as psv:
                    w_sb = [wp.tile([P, 3 * E], BF16, name=f"w_{dt}",
                                    tag=f"w_{dt}") for dt in range(DTI)]
                    qknext = [0]
                    for sc in range(S // XW):
                        x0 = sc * XW
                        xc = []
                        for dt in range(DTI):
                            if paired and sc == 1:
                                xt = xq23[dt]
                            else:
                                xt = xp.tile([P, XW], BF16, name=f"x{dt}",
                                             tag=f"x{dt}",
                                             bufs=1 if paired else 2)
                            nc.sync.dma_start(
                                xt[:, :], xT[dt * P:(dt + 1) * P, x0:x0 + XW])
                            xc.append(xt)
                            if paired and sc == 1:
                                nc.sync.dma_start(
                                    wq3[dt][:, :],
                                    wqkvT[dt * P:(dt + 1) * P, 0:E])
                            if sc == 0 and dt == 1:
                                # emitted after the first x/w tile pair so
                                # the first matmul's data leads the queue;
                                # the dummy exp pre-loads the ACT table
                                # (~2.7us) during the projections.
                                nc.sync.dma_start(
                                    mask_sb[:, :],
                                    maskT.rearrange("(t p) -> p t", p=P))
                                nc.scalar.activation(
                                    scr[:, :], mask_sb[:, :],
                                    mybir.ActivationFunctionType.Exp)
                            if sc == 0:
                                nc.sync.dma_start(
                                    w_sb[dt][:, :],
                                    wqkvT[dt * P:(dt + 1) * P, :])
                        if sc == 0:
                            for dv in range(HPC):
                                nc.sync.dma_start(
                                    wo_sb[dv][:, :],
                                    woT[dv * P:(dv + 1) * P, :])
                        for ch in range(XW // SB):
                            c0 = x0 + ch * SB
                            xs = ch * SB
                            # the last chunk's q projection is deferred into
                            # attention block 0 (its only reader is block 3's
                            # scores, ~150us later); block-0 heads otherwise
                            # idle the PE while exp paces them.
                            defer_q = paired and sc == 1
                            for half in range(2):
                                ets = (half * 2, half * 2 + 1)
                                ps = {}
                                # rotate over 6 PSUM tags so a half's first
                                # matmul never waits the previous half's
                                # eviction (4-tag rotation stalled ~432ns
                                # at every half boundary)
                                kinds = ("k",) if defer_q else ("q", "k")
                                for i, et in enumerate(ets):
                                    for kind in kinds:
                                        qknext[0] += 1
                                        tg = f"qk{qknext[0] % 6}"
                                        ps[(kind, et)] = psp.tile(
                                            [P, SB], F32, name=tg, tag=tg)
                                for dt in range(DTI):
                                    last = dt == DTI - 1 and not has_bias
                                    for et in ets:
                                        if not defer_q:
                                            nc.tensor.matmul(
                                                ps[("q", et)][:, :],
                                                w_sb[dt][:, et * P:(et + 1) * P],
                                                xc[dt][:, xs:xs + SB],
                                                start=(dt == 0), stop=last)
                                        nc.tensor.matmul(
                                            ps[("k", et)][:, :],
                                            w_sb[dt][:, E + et * P:E + (et + 1) * P],
                                            xc[dt][:, xs:xs + SB],
                                            start=(dt == 0), stop=last)
                                if has_bias:
                                    for et in ets:
                                        if not defer_q:
                                            nc.tensor.matmul(
                                                ps[("q", et)][:, :],
                                                bq_sb[0:1, et * P:(et + 1) * P],
                                                ones_row[0:1, 0:SB],
                                                start=False, stop=True)
                                        nc.tensor.matmul(
                                            ps[("k", et)][:, :],
                                            bk_sb[0:1, et * P:(et + 1) * P],
                                            ones_row[0:1, 0:SB],
                                            start=False, stop=True)
                                for et in ets:
                                    if not defer_q:
                                        nc.scalar.copy(qT[et][:, c0:c0 + SB],
                                                       ps[("q", et)][:, :])
                                    nc.vector.tensor_copy(
                                        kT[et][:, c0:c0 + SB],
                                        ps[("k", et)][:, :])
                            for sl in range(SB // P):
                                st_i = (c0 // P) + sl
                                vps = psv.tile([P, E], F32, name="vps")
                                for dt in range(DTI):
                                    nc.tensor.matmul(
                                        vps[:, :],
                                        xc[dt][:, xs + sl * P:xs + (sl + 1) * P],
                                        w_sb[dt][:, 2 * E:3 * E],
                                        start=(dt == 0),
                                        stop=(dt == DTI - 1 and not has_bias))
                                if has_bias:
                                    nc.tensor.matmul(
                                        vps[:, :], ones_rp[0:1, :],
                                        bv_sb[0:1, :], start=False, stop=True)
                                nc.scalar.copy(vv[st_i][:, :], vps[:, :])

                # ---------------- attention + out-projection ----------------
                # paired=True (trivial mask): two score matmuls share one
                # [128,1024] 2-bank PSUM tile and a single exp call -> 8
                # ScalarE ops per head instead of 16. With a nontrivial mask
                # the per-key bias needs one exp per key tile (paired=False).
                with nc.named_scope("attn"), \
                     tc.tile_pool(name="expp", bufs=12) as expp, \
                     tc.tile_pool(name="otnp", bufs=2) as otnp, \
                     tc.tile_pool(name="rbp", bufs=2) as rbp, \
                     tc.tile_pool(name="trp", bufs=2) as trp, \
                     tc.tile_pool(name="stage", bufs=5) as stagep, \
                     tc.tile_pool(name="stage2", bufs=5) as stagep2, \
                     tc.tile_pool(name="ps_sc", bufs=2, space="PSUM") as ps_sc, \
                     tc.tile_pool(name="ps_o", bufs=1 if paired else 2,
                                  space="PSUM") as ps_o, \
                     tc.tile_pool(name="ps_r", bufs=1, space="PSUM") as ps_r, \
                     tc.tile_pool(name="ps_y", bufs=2, space="PSUM") as ps_y:

                    def tree_add(pr, tr):
                        # elementwise [128,1024] pair-adds: 7 adds
                        # accumulate all 8 exp tiles, halves folded after.
                        for lvl in range(3):
                            w = 2 ** (lvl + 1)
                            if (pr + 1) % w == 0:
                                i = pr // w
                                a = (tr[("e", pr - w // 2)] if lvl == 0
                                     else tr[(lvl - 1, 2 * i)])
                                bb = (tr[("e", pr)] if lvl == 0
                                      else tr[(lvl - 1, 2 * i + 1)])
                                t = trp.tile([P, 2 * SB], BF16,
                                             name=f"u{lvl}", tag=f"u{lvl}")
                                nc.vector.tensor_add(t[:, :], a[:, :],
                                                     bb[:, :])
                                tr[(lvl, i)] = t

                    def emit_scores_exp(b, hh):
                        # scores+exp+tree for (block b, head hh), emitted
                        # ahead of an out-projection so ScalarE chews the
                        # exps while the PE runs the out-proj matmuls.
                        q0b = b * SB
                        exl = [None] * ST
                        tr = {}
                        for pr in range(ST // 2):
                            pssc = ps_sc.tile([P, 2 * SB], F32, name="pssc")
                            for half in range(2):
                                sk = 2 * pr + half
                                nc.tensor.matmul(
                                    pssc[:, half * SB:(half + 1) * SB],
                                    kT[hh][:, sk * P:(sk + 1) * P],
                                    qT[hh][:, q0b:q0b + SB],
                                    start=True, stop=True)
                            ext = expp.tile([P, 2 * SB], BF16, name="ext")
                            nc.scalar.activation(
                                ext[:, :], pssc[:, :],
                                mybir.ActivationFunctionType.Exp)
                            exl[2 * pr] = ext[:, 0:SB]
                            exl[2 * pr + 1] = ext[:, SB:2 * SB]
                            tr[("e", pr)] = ext
                            tree_add(pr, tr)
                        rsum = trp.tile([P, SB], BF16, name="rsum",
                                        tag="rsum")
                        big = tr[(2, 0)]
                        nc.vector.tensor_add(rsum[:, :], big[:, 0:SB],
                                             big[:, SB:2 * SB])
                        return exl, rsum

                    def emit_outproj(b, otn, eos, final=False):
                        for eo in eos:
                            yps = ps_y.tile([P, SB], F32, name="yps")
                            for dv in range(HPC):
                                nc.tensor.matmul(
                                    yps[:, :],
                                    wo_sb[dv][:, eo * P:(eo + 1) * P],
                                    otn[dv][:, :],
                                    start=(dv == 0), stop=(dv == HPC - 1))
                            if final and eo % 2 == 1:
                                # scalar is idle only in the final block's
                                # out-projection; alternate engines there
                                stg = stagep2.tile([P, SB], BF16, name="stg2")
                                nc.scalar.copy(stg[:, :], yps[:, :])
                            else:
                                stg = stagep.tile([P, SB], BF16, name="stg")
                                nc.vector.tensor_copy(stg[:, :], yps[:, :])
                            nc.sync.dma_start(
                                yT[eo * P:(eo + 1) * P, b * SB:(b + 1) * SB],
                                stg[:, :])

                    prev = None
                    preh0 = [None]   # pre-emitted (ex, rsum) for next blk h0
                    pending = None   # (ops, rsum, otn_tile) awaiting rps chain
                    for blk in range(NBLK):
                        q0 = blk * SB
                        otn = [otnp.tile([P, SB], BF16, name=f"otn{h}",
                                         tag=f"otn{h}")
                               for h in range(HPC)]
                        for h in range(HPC):
                            ex = [None] * ST
                            tr = {}
                            ops = ps_o.tile([P, SB], F32, name="ops")

                            def pv(j, ops=ops, ex=ex, h=h):
                                nc.tensor.matmul(
                                    ops[:, :],
                                    vv[j][:, h * P:(h + 1) * P],
                                    ex[j][:, :],
                                    start=(j == 0), stop=(j == ST - 1))

                            def tree_add(pr, tr):
                                # elementwise [128,1024] pair-adds: 7 adds
                                # accumulate all 8 exp tiles; halves folded
                                # after into the [128,512] key-sum.
                                for lvl in range(3):
                                    w = 2 ** (lvl + 1)
                                    if (pr + 1) % w == 0:
                                        i = pr // w
                                        a = (tr[("e", pr - w // 2)] if lvl == 0
                                             else tr[(lvl - 1, 2 * i)])
                                        bb = (tr[("e", pr)] if lvl == 0
                                              else tr[(lvl - 1, 2 * i + 1)])
                                        t = trp.tile([P, 2 * SB], BF16,
                                                     name=f"u{lvl}",
                                                     tag=f"u{lvl}")
                                        nc.vector.tensor_add(t[:, :], a[:, :],
                                                             bb[:, :])
                                        tr[(lvl, i)] = t

                            def flush_pending():
                                # rps chain of the PREVIOUS head, emitted
                                # after some of this head's scores so the PE
                                # never head-of-line blocks on the DVE tree.
                                nonlocal pending
                                if pending is None:
                                    return
                                p_ops, p_rsum, p_otn = pending
                                pending = None
                                rps = ps_r.tile([P, SB], F32, name="rps")
                                nc.tensor.matmul(
                                    rps[:, :], ones_sq[:, :], p_rsum[:, :],
                                    start=True, stop=True)
                                rb = rbp.tile([P, SB], F32, name="rb")
                                nc.vector.reciprocal_approx_fast(rb[:, :],
                                                                 rps[:, :])
                                nc.vector.tensor_mul(p_otn[:, :], p_ops[:, :],
                                                     rb[:, :])

                            if paired and h == 0 and preh0[0] is not None:
                                exp_, rsum = preh0[0]
                                preh0[0] = None
                                for j in range(ST):
                                    nc.tensor.matmul(
                                        ops[:, :],
                                        vv[j][:, h * P:(h + 1) * P],
                                        exp_[j][:, :],
                                        start=(j == 0), stop=(j == ST - 1))
                            elif paired:
                                for pr in range(ST // 2):
                                    pssc = ps_sc.tile([P, 2 * SB], F32,
                                                      name="pssc")
                                    for half in range(2):
                                        sk = 2 * pr + half
                                        nc.tensor.matmul(
                                            pssc[:, half * SB:(half + 1) * SB],
                                            kT[h][:, sk * P:(sk + 1) * P],
                                            qT[h][:, q0:q0 + SB],
                                            start=True, stop=True)
                                    ext = expp.tile([P, 2 * SB], BF16,
                                                    name="ext")
                                    nc.scalar.activation(
                                        ext[:, :], pssc[:, :],
                                        mybir.ActivationFunctionType.Exp)
                                    ex[2 * pr] = ext[:, 0:SB]
                                    ex[2 * pr + 1] = ext[:, SB:2 * SB]
                                    tr[("e", pr)] = ext
                                    tree_add(pr, tr)
                                    if blk <= 1:
                                        xoff = SB if blk == 0 else 0
                                        if pr == 0:
                                            qacc = ps_y.tile([P, SB], F32,
                                                             name="yps",
                                                             tag="yps")
                                        for dt in (2 * pr, 2 * pr + 1):
                                            nc.tensor.matmul(
                                                qacc[:, :],
                                                wq3[dt][:, h * P:(h + 1) * P],
                                                xq23[dt][:, xoff:xoff + SB],
                                                start=(dt == 0),
                                                stop=(dt == DTI - 1
                                                      and not has_bias),
                                                skip_group_check=True)
                                    for half in range(2):
                                        if 2 * pr + half >= 2:
                                            pv(2 * pr + half - 2)
                                pv(ST - 2)
                                pv(ST - 1)
                                if blk <= 1:
                                    if has_bias:
                                        nc.tensor.matmul(
                                            qacc[:, :],
                                            bq_sb[0:1, h * P:(h + 1) * P],
                                            ones_row[0:1, 0:SB],
                                            start=False, stop=True,
                                            skip_group_check=True)
                                    d0 = (3 if blk == 0 else 2) * SB
                                    nc.vector.tensor_copy(
                                        qT[h][:, d0:d0 + SB], qacc[:, :])
                                rsum = trp.tile([P, SB], BF16, name="rsum",
                                                tag="rsum")
                                big = tr[(2, 0)]
                                nc.vector.tensor_add(rsum[:, :],
                                                     big[:, 0:SB],
                                                     big[:, SB:2 * SB])
                            else:
                                for sk in range(ST):
                                    pssc = ps_sc.tile([P, SB], F32,
                                                      name="pssc")
                                    nc.tensor.matmul(
                                        pssc[:, :],
                                        kT[h][:, sk * P:(sk + 1) * P],
                                        qT[h][:, q0:q0 + SB],
                                        start=True, stop=True)
                                    ext = expp.tile([P, SB], BF16, name="ext")
                                    ex[sk] = ext
                                    nc.scalar.activation(
                                        ext[:, :], pssc[:, :],
                                        mybir.ActivationFunctionType.Exp,
                                        bias=mask_sb[:, sk:sk + 1], scale=1.0)
                                    for lvl in range(4):
                                        w = 2 ** (lvl + 1)
                                        if (sk + 1) % w == 0:
                                            i = sk // w
                                            a = (ex[sk - w // 2] if lvl == 0
                                                 else tr[(lvl - 1, 2 * i)])
                                            bb = (ex[sk] if lvl == 0
                                                  else tr[(lvl - 1, 2 * i + 1)])
                                            t = trp.tile([P, SB], BF16,
                                                         name=f"t{lvl}",
                                                         tag=f"t{lvl}")
                                            nc.vector.tensor_add(
                                                t[:, :], a[:, :], bb[:, :])
                                            tr[(lvl, i)] = t
                                    if sk >= 2:
                                        pv(sk - 2)
                                pv(ST - 2)
                                pv(ST - 1)
                                rsum = tr[(3, 0)]
                            pending = (ops, rsum, otn[h])
                            if h < HPC - 1:
                                flush_pending()
                        if prev is not None:
                            if paired and blk + 1 < NBLK:
                                preh0[0] = emit_scores_exp(blk + 1, 0)
                            flush_pending()
                            emit_outproj(prev[0], prev[1], range(DTI))
                        else:
                            flush_pending()
                        prev = (blk, otn)
                    flush_pending()
                    emit_outproj(*prev, range(DTI), final=True)

    nc.compile()
    return nc


_NC_CACHE = {}


def _get_nc(has_bias: bool, paired: bool):
    key = (has_bias, paired)
    if key not in _NC_CACHE:
        _NC_CACHE[key] = _build(has_bias, paired)
    return _NC_CACHE[key]


def kernel(hidden_states, attention_mask, Wq, bq, Wk, bk, Wv, bv, Wo, bo):
    hidden_states = np.asarray(hidden_states, dtype=np.float32)
    attention_mask = np.asarray(attention_mask, dtype=np.float32)
    Wq = np.asarray(Wq, dtype=np.float32)
    Wk = np.asarray(Wk, dtype=np.float32)
    Wv = np.asarray(Wv, dtype=np.float32)
    Wo = np.asarray(Wo, dtype=np.float32)
    bq = np.asarray(bq, dtype=np.float32)
    bk = np.asarray(bk, dtype=np.float32)
    bv = np.asarray(bv, dtype=np.float32)
    bo = np.asarray(bo, dtype=np.float32)

    has_bias = bool(np.any(bq) or np.any(bk) or np.any(bv))
    paired = bool(np.all(attention_mask == 1.0))
    nc = _get_nc(has_bias, paired)

    xT = [np.ascontiguousarray(hidden_states[b].T).astype(NPBF16)
          for b in range(B)]
    addmask = [np.ascontiguousarray((1.0 - attention_mask[b]) * MASK_MIN)
               for b in range(B)]
    in_maps = []
    for c in range(N_CORES):
        b, g = c // 4, c % 4
        sl = slice(g * E, (g + 1) * E)
        wqkv = np.concatenate(
            [Wq[sl, :].T * SCALE, Wk[sl, :].T, Wv[sl, :].T], axis=1)
        im = {
            "xT": xT[b],
            "wqkvT": np.ascontiguousarray(wqkv).astype(NPBF16),
            "woT": np.ascontiguousarray(Wo[:, sl].T).astype(NPBF16),
            "maskT": addmask[b],
        }
        if has_bias:
            im["ones1"] = np.ones(SB, dtype=NPBF16)
            im["bq"] = np.ascontiguousarray(bq[sl] * SCALE).astype(NPBF16)
            im["bk"] = np.ascontiguousarray(bk[sl]).astype(NPBF16)
            im["bv"] = np.ascontiguousarray(bv[sl]).astype(NPBF16)
        in_maps.append(im)

    res = bass_utils.run_bass_kernel_spmd(
        nc, in_maps, core_ids=list(range(N_CORES)),
        trace=bool(int(os.environ.get("BASS_KERNEL_TRACE", "0"))))
    kernel.last_results = res

    out = np.empty((B, S, D), dtype=np.float32)
    for b in range(B):
        acc = res.results[b * 4]["yT"].astype(np.float32)
        for g in range(1, 4):
            acc += res.results[b * 4 + g]["yT"].astype(np.float32)
        out[b] = acc.T + bo
    return out


# revision 33
# speedup vs baseline: 1.0052x; 1.0052x over previous
"""LlamaAttention (B=2, S=2048, D=2048, H=16) on 8 Trainium2 NeuronCores.

Sharding: batch x head-group. Core c handles batch b = c // 4 and head group
g = c % 4 (4 heads of 128 dims each -> a 512-wide slice of q/k/v space).
Each core computes q/k/v projections for its slice, attention for its 4
heads, and a partial out-projection (contracting only its 512 dv dims).
Host sums the 4 partials per batch and adds the output bias.

Design (measured ~365us vs the 694us v1 baseline; rel err ~5.4e-3 vs 2e-2):
  - All matmul data is bf16 (same PE rate as fp32r, half the DMA/SBUF);
    accumulation is fp32 in PSUM, softmax denominators are exact fp32.
  - One x pass computes q/k/v: packed [D,3E] weight tiles (3KB DMA lines),
    x in [128,1024] tiles (2KB lines), chunk-0 x DMAs interleaved with
    weight DMAs so the first matmul issues after ~2 tile loads. q/k
    accumulate dt-major in 4 PSUM banks (two et-halves); SCALE is folded
    into Wq on the host.
  - Attention uses the transposed softmax layout (keys on partitions):
    scores^T tiles -> exp on ScalarE straight out of PSUM. With the
    all-ones mask two score matmuls share one [128,1024] 2-bank PSUM tile
    and a single exp (8 ScalarE ops/head); a nontrivial mask switches to
    the per-key-tile variant with the mask as the activation's
    per-partition bias (exact general masking).
  - Row sums (softmax denominator) ride a 7-op elementwise [128,1024]
    bf16 add tree on the DVE; one all-ones [128,128] stationary matmul
    per head then partition-reduces AND broadcasts the sum across all
    128 partitions in one shot, so normalization is reciprocal_approx_fast
    + one tensor_mul (no GpSimd, no per-partition broadcast).
  - scores/PV matmuls interleave at key-tile granularity; the previous
    block's out-projection is emitted between blocks. Engine assignment:
    exp + q/v evictions on ScalarE, k/y evictions + tree + normalize on
    DVE; the final block's out-projection alternates evict engines.
  - q/k PSUM accumulators rotate over 6 tags so a chunk-half's first
    matmul never waits the previous half's eviction.
  - The q projections of s-chunks 3 and 2 are deferred into attention
    blocks 0 and 1 (their readers - blocks 3 and 2's scores - run much
    later): those heads otherwise idle the PE while exp paces them, so
    the 128 deferred matmuls ride in the idle slots, shortening the
    projection phase by ~28us. Each accumulator borrows the out-proj
    PSUM tag during a window where it is provably idle (out-proj of
    block b-1 executes only after block b's heads). The operands are
    dedicated persistent SBUF copies: the x superchunk-1 copy doubles as
    the projection's own input, so the xp pool drops to single-buffered
    and superchunk 1 prefetches with no write-after-read wait at all.
  - Head 0 of blocks 2 and 3 is pre-computed: its scores/exp/tree are
    emitted ahead of the preceding block's out-projection, so ScalarE
    chews those exps during the otherwise scalar-idle out-proj window
    and the head body reduces to a pure PV burst.
  - Known wall: the attention phase rides the chip power limit. Tighter
    interleavings (out-proj inside heads, deferred r-chain) measurably
    tripped a chip-wide ~1.2x downclock (P0: matmuls 379->454ns, exp
    1112->1334ns), netting slower kernels; the phase-alternating
    schedule stays at 2.4GHz and is within ~4% of the power-constrained
    optimum. ScalarE exp (~9us/head) vs PE (~7us/head + out-proj share)
    are nearly balanced there.
"""

import os
import numpy as np
import ml_dtypes

import concourse.bass as bass
import concourse.tile as tile
from concourse import bacc, mybir
from concourse import bass_utils

B, S, D = 2, 2048, 2048
NH, HD = 16, 128
N_CORES = 8
HPC = 4                      # heads per core
E = HPC * HD                 # 512: per-core q/k/v width
SCALE = float(HD) ** -0.5
F32 = mybir.dt.float32
BF16 = mybir.dt.bfloat16
NPBF16 = ml_dtypes.bfloat16

P = 128                      # partition tile
ST = S // P                  # 16 s partition-tiles
DTI = D // P                 # 16 d partition-tiles
SB = 512                     # matmul moving-dim block
NBLK = S // SB               # 4 s blocks
MASK_MIN = float(np.finfo(np.float32).min)

MM_DT = BF16                 # for test.py's printout


def _build(has_bias: bool, paired: bool):
    nc = bacc.Bacc("TRN2", target_bir_lowering=False, debug=False,
                   num_devices=N_CORES)

    xT = nc.dram_tensor("xT", [D, S], BF16, kind="ExternalInput").ap()
    wqkvT = nc.dram_tensor("wqkvT", [D, 3 * E], BF16,
                           kind="ExternalInput").ap()
    woT = nc.dram_tensor("woT", [E, D], BF16, kind="ExternalInput").ap()
    maskT = nc.dram_tensor("maskT", [S], F32, kind="ExternalInput").ap()
    if has_bias:
        ones1 = nc.dram_tensor("ones1", [SB], BF16, kind="ExternalInput").ap()
        bqd = nc.dram_tensor("bq", [E], BF16, kind="ExternalInput").ap()
        bkd = nc.dram_tensor("bk", [E], BF16, kind="ExternalInput").ap()
        bvd = nc.dram_tensor("bv", [E], BF16, kind="ExternalInput").ap()
    yT = nc.dram_tensor("yT", [D, S], BF16, kind="ExternalOutput").ap()

    with tile.TileContext(nc) as tc:
        with tc.tile_pool(name="persist", bufs=1) as persist:
            qT = [persist.tile([P, S], BF16, name=f"qT{i}", tag=f"qT{i}")
                  for i in range(HPC)]
            kT = [persist.tile([P, S], BF16, name=f"kT{i}", tag=f"kT{i}")
                  for i in range(HPC)]
            mask_sb = persist.tile([P, ST], F32, name="mask_sb", tag="mask")
            ones_sq = persist.tile([P, P], BF16, name="ones_sq", tag="onesq")
            scr = persist.tile([P, ST], BF16, name="scr", tag="scr")
            nc.vector.memset(ones_sq[:, :], 1.0)
            if has_bias:
                ones_row = persist.tile([1, SB], BF16, name="ones_row",
                                        tag="onesr")
                nc.sync.dma_start(ones_row[:, :],
                                  ones1.rearrange("(a e) -> a e", a=1))
                ones_rp = persist.tile([1, P], BF16, name="ones_rp",
                                       tag="onesrp")
                nc.sync.dma_start(ones_rp[:, :],
                                  ones1[0:P].rearrange("(a e) -> a e", a=1))
                bq_sb = persist.tile([1, E], BF16, name="bq_sb", tag="bq")
                bk_sb = persist.tile([1, E], BF16, name="bk_sb", tag="bk")
                bv_sb = persist.tile([1, E], BF16, name="bv_sb", tag="bv")
                nc.sync.dma_start(bq_sb[:, :], bqd.rearrange("(a e) -> a e", a=1))
                nc.sync.dma_start(bk_sb[:, :], bkd.rearrange("(a e) -> a e", a=1))
                nc.sync.dma_start(bv_sb[:, :], bvd.rearrange("(a e) -> a e", a=1))

            with tc.tile_pool(name="vvp", bufs=1) as vvp, \
                 tc.tile_pool(name="wop", bufs=1) as wop:
                vv = [vvp.tile([P, E], BF16, name=f"v{i}", tag=f"v{i}")
                      for i in range(ST)]
                wo_sb = [wop.tile([P, D], BF16, name=f"wo{i}", tag=f"wo{i}")
                         for i in range(HPC)]
                # dedicated copies of the deferred-q operands (w q-columns;
                # x superchunk 1 lives here outright and the projections read
                # it in place, so xp needs no double buffering for it)
                if paired:
                    wq3 = [vvp.tile([P, E], BF16, name=f"wq3_{i}",
                                    tag=f"wq3_{i}") for i in range(DTI)]
                    xq23 = [vvp.tile([P, 2 * SB], BF16, name=f"xq23_{i}",
                                     tag=f"xq23_{i}") for i in range(DTI)]

                # ---------------- projections: one x pass ----------------
                # x streams once in [128,1024] tiles (2KB DMA lines); the
                # packed wqkv weight tiles are [128,1536] (3KB lines).
                # Chunk-0 x DMAs interleave with weight DMAs so the first
                # matmul starts after ~2 tile loads, not the full weight set.
                XW = 2 * SB
                with nc.named_scope("proj"), \
                     tc.tile_pool(name="wqkv", bufs=1) as wp, \
                     tc.tile_pool(name="xp", bufs=2) as xp, \
                     tc.tile_pool(name="ps_p", bufs=1, space="PSUM") as psp, \
                     tc.tile_pool(name="ps_v", bufs=2, space="PSUM") as psv:
                    w_sb = [wp.tile([P, 3 * E], BF16, name=f"w_{dt}",
                                    tag=f"w_{dt}") for dt in range(DTI)]
                    qknext = [0]
                    for sc in range(S // XW):
                        x0 = sc * XW
                        xc = []
                        for dt in range(DTI):
                            if paired and sc == 1:
                                xt = xq23[dt]
                            else:
                                xt = xp.tile([P, XW], BF16, name=f"x{dt}",
                                             tag=f"x{dt}",
                                             bufs=1 if paired else 2)
                            nc.sync.dma_start(
                                xt[:, :], xT[dt * P:(dt + 1) * P, x0:x0 + XW])
                            xc.append(xt)
                            if paired and sc == 1:
                                nc.sync.dma_start(
                                    wq3[dt][:, :],
                                    wqkvT[dt * P:(dt + 1) * P, 0:E])
                            if sc == 0 and dt == 1:
                                # emitted after the first x/w tile pair so
                                # the first matmul's data leads the queue;
                                # the dummy exp pre-loads the ACT table
                                # (~2.7us) during the projections.
                                nc.sync.dma_start(
                                    mask_sb[:, :],
                                    maskT.rearrange("(t p) -> p t", p=P))
                                nc.scalar.activation(
                                    scr[:, :], mask_sb[:, :],
                                    mybir.ActivationFunctionType.Exp)
                            if sc == 0:
                                nc.sync.dma_start(
                                    w_sb[dt][:, :],
                                    wqkvT[dt * P:(dt + 1) * P, :])
                        if sc == 0:
                            for dv in range(HPC):
                                nc.sync.dma_start(
                                    wo_sb[dv][:, :],
                                    woT[dv * P:(dv + 1) * P, :])
                        for ch in range(XW // SB):
                            c0 = x0 + ch * SB
                            xs = ch * SB
                            # the last chunk's q projection is deferred into
                            # attention block 0 (its only reader is block 3's
                            # scores, ~150us later); block-0 heads otherwise
                            # idle the PE while exp paces them.
                            defer_q = paired and sc == 1
                            for half in range(2):
                                ets = (half * 2, half * 2 + 1)
                                ps = {}
                                # rotate over 6 PSUM tags so a half's first
                                # matmul never waits the previous half's
                                # eviction (4-tag rotation stalled ~432ns
                                # at every half boundary)
                                kinds = ("k",) if defer_q else ("q", "k")
                                for i, et in enumerate(ets):
                                    for kind in kinds:
                                        qknext[0] += 1
                                        tg = f"qk{qknext[0] % 6}"
                                        ps[(kind, et)] = psp.tile(
                                            [P, SB], F32, name=tg, tag=tg)
                                for dt in range(DTI):
                                    last = dt == DTI - 1 and not has_bias
                                    for et in ets:
                                        if not defer_q:
                                            nc.tensor.matmul(
                                                ps[("q", et)][:, :],
                                                w_sb[dt][:, et * P:(et + 1) * P],
                                                xc[dt][:, xs:xs + SB],
                                                start=(dt == 0), stop=last)
                                        nc.tensor.matmul(
                                            ps[("k", et)][:, :],
                                            w_sb[dt][:, E + et * P:E + (et + 1) * P],
                                            xc[dt][:, xs:xs + SB],
                                            start=(dt == 0), stop=last)
                                if has_bias:
                                    for et in ets:
                                        if not defer_q:
                                            nc.tensor.matmul(
                                                ps[("q", et)][:, :],
                                                bq_sb[0:1, et * P:(et + 1) * P],
                                                ones_row[0:1, 0:SB],
                                                start=False, stop=True)
                                        nc.tensor.matmul(
                                            ps[("k", et)][:, :],
                                            bk_sb[0:1, et * P:(et + 1) * P],
                                            ones_row[0:1, 0:SB],
                                            start=False, stop=True)
                                for et in ets:
                                    if not defer_q:
                                        nc.scalar.copy(qT[et][:, c0:c0 + SB],
                                                       ps[("q", et)][:, :])
                                    nc.vector.tensor_copy(
                                        kT[et][:, c0:c0 + SB],
                                        ps[("k", et)][:, :])
                            for sl in range(SB // P):
                                st_i = (c0 // P) + sl
                                vps = psv.tile([P, E], F32, name="vps")
                                for dt in range(DTI):
                                    nc.tensor.matmul(
                                        vps[:, :],
                                        xc[dt][:, xs + sl * P:xs + (sl + 1) * P],
                                        w_sb[dt][:, 2 * E:3 * E],
                                        start=(dt == 0),
                                        stop=(dt == DTI - 1 and not has_bias))
                                if has_bias:
                                    nc.tensor.matmul(
                                        vps[:, :], ones_rp[0:1, :],
                                        bv_sb[0:1, :], start=False, stop=True)
                                nc.scalar.copy(vv[st_i][:, :], vps[:, :])

                # ---------------- attention + out-projection ----------------
                # paired=True (trivial mask): two score matmuls share one
                # [128,1024] 2-bank PSUM tile and a single exp call -> 8
                # ScalarE ops per head instead of 16. With a nontrivial mask
                # the per-key bias needs one exp per key tile (paired=False).
                with nc.named_scope("attn"), \
                     tc.tile_pool(name="expp", bufs=12) as expp, \
                     tc.tile_pool(name="otnp", bufs=2) as otnp, \
                     tc.tile_pool(name="rbp", bufs=2) as rbp, \
                     tc.tile_pool(name="trp", bufs=2) as trp, \
                     tc.tile_pool(name="stage", bufs=5) as stagep, \
                     tc.tile_pool(name="stage2", bufs=5) as stagep2, \
                     tc.tile_pool(name="ps_sc", bufs=2, space="PSUM") as ps_sc, \
                     tc.tile_pool(name="ps_o", bufs=1 if paired else 2,
                                  space="PSUM") as ps_o, \
                     tc.tile_pool(name="ps_r", bufs=1, space="PSUM") as ps_r, \
                     tc.tile_pool(name="ps_y", bufs=2, space="PSUM") as ps_y:

                    def tree_add(pr, tr):
                        # elementwise [128,1024] pair-adds: 7 adds
                        # accumulate all 8 exp tiles, halves folded after.
                        for lvl in range(3):
                            w = 2 ** (lvl + 1)
                            if (pr + 1) % w == 0:
                                i = pr // w
                                a = (tr[("e", pr - w // 2)] if lvl == 0
                                     else tr[(lvl - 1, 2 * i)])
                                bb = (tr[("e", pr)] if lvl == 0
                                      else tr[(lvl - 1, 2 * i + 1)])
                                t = trp.tile([P, 2 * SB], BF16,
                                             name=f"u{lvl}", tag=f"u{lvl}")
                                nc.vector.tensor_add(t[:, :], a[:, :],
                                                     bb[:, :])
                                tr[(lvl, i)] = t

                    def emit_scores_exp(b, hh):
                        # scores+exp+tree for (block b, head hh), emitted
                        # ahead of an out-projection so ScalarE chews the
                        # exps while the PE runs the out-proj matmuls.
                        q0b = b * SB
                        exl = [None] * ST
                        tr = {}
                        for pr in range(ST // 2):
                            pssc = ps_sc.tile([P, 2 * SB], F32, name="pssc")
                            for half in range(2):
                                sk = 2 * pr + half
                                nc.tensor.matmul(
                                    pssc[:, half * SB:(half + 1) * SB],
                                    kT[hh][:, sk * P:(sk + 1) * P],
                                    qT[hh][:, q0b:q0b + SB],
                                    start=True, stop=True)
                            ext = expp.tile([P, 2 * SB], BF16, name="ext")
                            nc.scalar.activation(
                                ext[:, :], pssc[:, :],
                                mybir.ActivationFunctionType.Exp)
                            exl[2 * pr] = ext[:, 0:SB]
                            exl[2 * pr + 1] = ext[:, SB:2 * SB]
                            tr[("e", pr)] = ext
                            tree_add(pr, tr)
                        rsum = trp.tile([P, SB], BF16, name="rsum",
                                        tag="rsum")
                        big = tr[(2, 0)]
                        nc.vector.tensor_add(rsum[:, :], big[:, 0:SB],
                                             big[:, SB:2 * SB])
                        return exl, rsum

                    def emit_outproj(b, otn, eos, final=False):
                        for eo in eos:
                            yps = ps_y.tile([P, SB], F32, name="yps")
                            for dv in range(HPC):
                                nc.tensor.matmul(
                                    yps[:, :],
                                    wo_sb[dv][:, eo * P:(eo + 1) * P],
                                    otn[dv][:, :],
                                    start=(dv == 0), stop=(dv == HPC - 1))
                            if final and eo % 2 == 1:
                                # scalar is idle only in the final block's
                                # out-projection; alternate engines there
                                stg = stagep2.tile([P, SB], BF16, name="stg2")
                                nc.scalar.copy(stg[:, :], yps[:, :])
                            else:
                                stg = stagep.tile([P, SB], BF16, name="stg")
                                nc.vector.tensor_copy(stg[:, :], yps[:, :])
                            nc.sync.dma_start(
                                yT[eo * P:(eo + 1) * P, b * SB:(b + 1) * SB],
                                stg[:, :])

                    prev = None
                    preh0 = [None]   # pre-emitted (ex, rsum) for next blk h0
                    pending = None   # (ops, rsum, otn_tile) awaiting rps chain
                    for blk in range(NBLK):
                        q0 = blk * SB
                        otn = [otnp.tile([P, SB], BF16, name=f"otn{h}",
                                         tag=f"otn{h}")
                               for h in range(HPC)]
                        for h in range(HPC):
                            ex = [None] * ST
                            tr = {}
                            ops = ps_o.tile([P, SB], F32, name="ops")

                            def pv(j, ops=ops, ex=ex, h=h):
                                nc.tensor.matmul(
                                    ops[:, :],
                                    vv[j][:, h * P:(h + 1) * P],
                                    ex[j][:, :],
                                    start=(j == 0), stop=(j == ST - 1))

                            def tree_add(pr, tr):
                                # elementwise [128,1024] pair-adds: 7 adds
                                # accumulate all 8 exp tiles; halves folded
                                # after into the [128,512] key-sum.
                                for lvl in range(3):
                                    w = 2 ** (lvl + 1)
                                    if (pr + 1) % w == 0:
                                        i = pr // w
                                        a = (tr[("e", pr - w // 2)] if lvl == 0
                                             else tr[(lvl - 1, 2 * i)])
                                        bb = (tr[("e", pr)] if lvl == 0
                                              else tr[(lvl - 1, 2 * i + 1)])
                                        t = trp.tile([P, 2 * SB], BF16,
                                                     name=f"u{lvl}",
                                                     tag=f"u{lvl}")
                                        nc.vector.tensor_add(t[:, :], a[:, :],
                                                             bb[:, :])
                                        tr[(lvl, i)] = t

                            def flush_pending():
                                # rps chain of the PREVIOUS head, emitted
                                # after some of this head's scores so the PE
                                # never head-of-line blocks on the DVE tree.
                                nonlocal pending
                                if pending is None:
                                    return
                                p_ops, p_rsum, p_otn = pending
                                pending = None
                                rps = ps_r.tile([P, SB], F32, name="rps")
                                nc.tensor.matmul(
                                    rps[:, :], ones_sq[:, :], p_rsum[:, :],
                                    start=True, stop=True)
                                rb = rbp.tile([P, SB], F32, name="rb")
                                nc.vector.reciprocal_approx_fast(rb[:, :],
                                                                 rps[:, :])
                                nc.vector.tensor_mul(p_otn[:, :], p_ops[:, :],
                                                     rb[:, :])

                            if paired and h == 0 and preh0[0] is not None:
                                exp_, rsum = preh0[0]
                                preh0[0] = None
                                for j in range(ST):
                                    nc.tensor.matmul(
                                        ops[:, :],
                                        vv[j][:, h * P:(h + 1) * P],
                                        exp_[j][:, :],
                                        start=(j == 0), stop=(j == ST - 1))
                            elif paired:
                                for pr in range(ST // 2):
                                    pssc = ps_sc.tile([P, 2 * SB], F32,
                                                      name="pssc")
                                    for half in range(2):
                                        sk = 2 * pr + half
                                        nc.tensor.matmul(
                                            pssc[:, half * SB:(half + 1) * SB],
                                            kT[h][:, sk * P:(sk + 1) * P],
                                            qT[h][:, q0:q0 + SB],
                                            start=True, stop=True)
                                    ext = expp.tile([P, 2 * SB], BF16,
                                                    name="ext")
                                    nc.scalar.activation(
                                        ext[:, :], pssc[:, :],
                                        mybir.ActivationFunctionType.Exp)
                                    ex[2 * pr] = ext[:, 0:SB]
                                    ex[2 * pr + 1] = ext[:, SB:2 * SB]
                                    tr[("e", pr)] = ext
                                    tree_add(pr, tr)
                                    if blk <= 1:
                                        xoff = SB if blk == 0 else 0
                                        if pr == 0:
                                            qacc = ps_y.tile([P, SB], F32,
                                                             name="yps",
                                                             tag="yps")
                                        for dt in (2 * pr, 2 * pr + 1):
                                            nc.tensor.matmul(
                                                qacc[:, :],
                                                wq3[dt][:, h * P:(h + 1) * P],
                                                xq23[dt][:, xoff:xoff + SB],
                                                start=(dt == 0),
                                                stop=(dt == DTI - 1
                                                      and not has_bias),
                                                skip_group_check=True)
                                    for half in range(2):
                                        if 2 * pr + half >= 2:
                                            pv(2 * pr + half - 2)
                                pv(ST - 2)
                                pv(ST - 1)
                                if blk <= 1:
                                    if has_bias:
                                        nc.tensor.matmul(
                                            qacc[:, :],
                                            bq_sb[0:1, h * P:(h + 1) * P],
                                            ones_row[0:1, 0:SB],
                                            start=False, stop=True,
                                            skip_group_check=True)
                                    d0 = (3 if blk == 0 else 2) * SB
                                    nc.vector.tensor_copy(
                                        qT[h][:, d0:d0 + SB], qacc[:, :])
                                rsum = trp.tile([P, SB], BF16, name="rsum",
                                                tag="rsum")
                                big = tr[(2, 0)]
                                nc.vector.tensor_add(rsum[:, :],
                                                     big[:, 0:SB],
                                                     big[:, SB:2 * SB])
                            else:
                                for sk in range(ST):
                                    pssc = ps_sc.tile([P, SB], F32,
                                                      name="pssc")
                                    nc.tensor.matmul(
                                        pssc[:, :],
                                        kT[h][:, sk * P:(sk + 1) * P],
                                        qT[h][:, q0:q0 + SB],
                                        start=True, stop=True)
                                    ext = expp.tile([P, SB], BF16, name="ext")
                                    ex[sk] = ext
                                    nc.scalar.activation(
                                        ext[:, :], pssc[:, :],
                                        mybir.ActivationFunctionType.Exp,
                                        bias=mask_sb[:, sk:sk + 1], scale=1.0)
                                    for lvl in range(4):
                                        w = 2 ** (lvl + 1)
                                        if (sk + 1) % w == 0:
                                            i = sk // w
                                            a = (ex[sk - w // 2] if lvl == 0
                                                 else tr[(lvl - 1, 2 * i)])
                                            bb = (ex[sk] if lvl == 0
                                                  else tr[(lvl - 1, 2 * i + 1)])
                                            t = trp.tile([P, SB], BF16,
                                                         name=f"t{lvl}",
                                                         tag=f"t{lvl}")
                                            nc.vector.tensor_add(
                                                t[:, :], a[:, :], bb[:, :])
                                            tr[(lvl, i)] = t
                                    if sk >= 2:
                                        pv(sk - 2)
                                pv(ST - 2)
                                pv(ST - 1)
                                rsum = tr[(3, 0)]
                            pending = (ops, rsum, otn[h])
                            flush_pending()
                        if prev is not None:
                            if paired and blk + 1 < NBLK:
                                preh0[0] = emit_scores_exp(blk + 1, 0)
                            emit_outproj(prev[0], prev[1], range(DTI))
                            flush_pending()
                        prev = (blk, otn)
                    flush_pending()
                    emit_outproj(*prev, range(DTI), final=True)

    nc.compile()
    return nc


_NC_CACHE = {}


def _get_nc(has_bias: bool, paired: bool):
    key = (has_bias, paired)
    if key not in _NC_CACHE:
        _NC_CACHE[key] = _build(has_bias, paired)
    return _NC_CACHE[key]


def kernel(hidden_states, attention_mask, Wq, bq, Wk, bk, Wv, bv, Wo, bo):
    hidden_states = np.asarray(hidden_states, dtype=np.float32)
    attention_mask = np.asarray(attention_mask, dtype=np.float32)
    Wq = np.asarray(Wq, dtype=np.float32)
    Wk = np.asarray(Wk, dtype=np.float32)
    Wv = np.asarray(Wv, dtype=np.float32)
    Wo = np.asarray(Wo, dtype=np.float32)
    bq = np.asarray(bq, dtype=np.float32)
    bk = np.asarray(bk, dtype=np.float32)
    bv = np.asarray(bv, dtype=np.float32)
    bo = np.asarray(bo, dtype=np.float32)

    has_bias = bool(np.any(bq) or np.any(bk) or np.any(bv))
    paired = bool(np.all(attention_mask == 1.0))
    nc = _get_nc(has_bias, paired)

    xT = [np.ascontiguousarray(hidden_states[b].T).astype(NPBF16)
          for b in range(B)]
    addmask = [np.ascontiguousarray((1.0 - attention_mask[b]) * MASK_MIN)
               for b in range(B)]
    in_maps = []
    for c in range(N_CORES):
        b, g = c // 4, c % 4
        sl = slice(g * E, (g + 1) * E)
        wqkv = np.concatenate(
            [Wq[sl, :].T * SCALE, Wk[sl, :].T, Wv[sl, :].T], axis=1)
        im = {
            "xT": xT[b],
            "wqkvT": np.ascontiguousarray(wqkv).astype(NPBF16),
            "woT": np.ascontiguousarray(Wo[:, sl].T).astype(NPBF16),
            "maskT": addmask[b],
        }
        if has_bias:
            im["ones1"] = np.ones(SB, dtype=NPBF16)
            im["bq"] = np.ascontiguousarray(bq[sl] * SCALE).astype(NPBF16)
            im["bk"] = np.ascontiguousarray(bk[sl]).astype(NPBF16)
            im["bv"] = np.ascontiguousarray(bv[sl]).astype(NPBF16)
        in_maps.append(im)

    res = bass_utils.run_bass_kernel_spmd(
        nc, in_maps, core_ids=list(range(N_CORES)),
        trace=bool(int(os.environ.get("BASS_KERNEL_TRACE", "0"))))
    kernel.last_results = res

    out = np.empty((B, S, D), dtype=np.float32)
    for b in range(B):
        acc = res.results[b * 4]["yT"].astype(np.float32)
        for g in range(1, 4):
            acc += res.results[b * 4 + g]["yT"].astype(np.float32)
        out[b] = acc.T + bo
    return out
